# revision 1
# baseline (speedup 1.0000x reference)
"""Trainium2 Bass kernel for nn_MultiHeadAttention_72069551227273.

Reference computation (B=2, S=2048, D_MODEL=D_EMB=1024, H=16, d_k=64):
    q_p = q @ W_in + b_in                    (shared input projection)
    qh  = heads(q_p @ Wq + bq)               (per-head projections)
    s   = qh @ kh^T / sqrt(d_k), causal-masked softmax
    out = (attn @ vh, concat heads) @ Wo + bo

Sharding: 8 cores = 2 (batch) x 4 (head groups of 4 heads / 256 emb cols).
Per core, the input and head projections are algebraically fused:
    Q = q @ (W_in @ Wq_slice) + (b_in @ Wq_slice + bq_slice)
with the weight combine computed on device.  Everything on-chip is kept in
a transposed layout ([feature, seq]) so the only activation "transposes"
are free host-side relayouts of the kernel inputs during sharding.
Softmax is computed without max-subtraction (scores are O(1) for this
problem) as exp(s/8), with the denominator obtained for free by
augmenting V with a ones column in the attn @ V matmul.  The masked-block
structure is resolved at trace time: fully-masked score blocks are
skipped, full blocks need no masking, and diagonal blocks get a
triangular mask generated on-chip.  After the per-core output projection
a 4-way ReduceScatter produces each core's row shard of the final output;
bo is added during the post-collective DMA (accumulating SWDGE copy).
"""

import sys

sys.path.append("/opt/trn_rl_repo")

import math
from contextlib import ExitStack

import numpy as np

import concourse.bass as bass
import concourse.bacc as bacc
import concourse.mybir as mybir
import concourse.tile as tile
from concourse import bass_utils
from concourse.bass_interp import get_hw_module
from concourse.masks import make_identity

# problem dims
B, S, DM, DE, H, DK = 2, 2048, 1024, 1024, 16, 64
N_CORES = 8
GROUPS = [[0, 1, 2, 3], [4, 5, 6, 7]]
P = 128                      # partitions
QC = 512                     # q chunk (psum bank width in fp32)
KB = 128                     # k block (scores^T partition block)

F32 = mybir.dt.float32
BF16 = mybir.dt.bfloat16
F32R = mybir.dt.float32r

FULL, TRI, GEN, SKIP = 0, 1, 2, 3


def make_plan(mask_np, s=S, qc=QC, kb=KB):
    """Classify scores^T blocks [kb x qc] from the (B, S, S) 0/1 mask.

    Returns (blocks, n_gen_tiles, gen_tiles_per_batch):
      blocks[iqc] = list of (ikb, mode, arg)
    """
    nqc, nkb = s // qc, s // kb
    m = np.asarray(mask_np) != 0          # [B, S(q), S(k)] True = attend
    tril = np.tril(np.ones((s, s), bool))
    causal = all(np.array_equal(m[b], tril) for b in range(m.shape[0]))
    blocks = []
    if causal:
        for iqc in range(nqc):
            row = []
            for ikb in range(nkb):
                if (ikb + 1) * kb <= iqc * qc:
                    row.append((ikb, FULL, 0))
                elif ikb * kb < (iqc + 1) * qc:
                    row.append((ikb, TRI, (ikb * kb - iqc * qc) // kb))
                # else fully masked -> skip
            blocks.append(row)
        return blocks, 0, None

    # general path: per-block classification, unioned across batches
    nb = m.shape[0]
    # every query row must attend to >= 1 key (else softmax semantics differ)
    assert m.any(axis=-1).all(), "fully-masked query rows unsupported"
    gen_tiles = [[] for _ in range(nb)]
    for iqc in range(nqc):
        row = []
        for ikb in range(nkb):
            sub = m[:, iqc * qc:(iqc + 1) * qc, ikb * kb:(ikb + 1) * kb]
            if sub.all():
                row.append((ikb, FULL, 0))
            elif not sub.any():
                continue
            else:
                idx = len(gen_tiles[0])
                for b in range(nb):
                    gen_tiles[b].append(sub[b].T.astype(np.int32))  # [kb, qc]
                row.append((ikb, GEN, idx))
        blocks.append(row)
    n_gen = len(gen_tiles[0])
    gt = [np.stack(g) if n_gen else np.zeros((1, kb, qc), np.int32)
          for g in gen_tiles]
    return blocks, n_gen, gt


def build_mha(blocks, n_gen, *, s=S, dm=DM, de=DE, dh=None, mm="bf16",
              collective=True):
    """Trace the per-core MHA program.  dh = per-core emb slice (256)."""
    if dh is None:
        dh = DE // 4
    nqc, nkb, ndm, nde = s // QC, s // KB, dm // P, de // P
    ndh = dh // P            # dout chunks (2)
    hloc = dh // DK          # heads per core (4)
    out_rows = s // 4        # reduce-scatter shard rows per core

    st = BF16 if mm == "bf16" else F32          # storage dtype for matmul inputs
    def mmv(ap):                                 # matmul operand view
        return ap.bitcast(F32R) if mm == "f32r" else ap

    nc = bacc.Bacc("TRN2", target_bir_lowering=False, debug=False,
                   num_devices=N_CORES)

    # ---- kernel I/O (per core) ----
    qT = nc.dram_tensor("qT", [dm, s], F32, kind="ExternalInput")
    kT = nc.dram_tensor("kT", [dm, s], F32, kind="ExternalInput")
    vT = nc.dram_tensor("vT", [dm, s], F32, kind="ExternalInput")
    w_inT = nc.dram_tensor("w_inT", [de, dm], F32, kind="ExternalInput")
    wq = nc.dram_tensor("wq", [de, dh], F32, kind="ExternalInput")
    wk = nc.dram_tensor("wk", [de, dh], F32, kind="ExternalInput")
    wv = nc.dram_tensor("wv", [de, dh], F32, kind="ExternalInput")
    wo = nc.dram_tensor("wo", [dh, dm], F32, kind="ExternalInput")
    b_in = nc.dram_tensor("b_in", [de], F32, kind="ExternalInput")
    bq = nc.dram_tensor("bq", [dh], F32, kind="ExternalInput")
    bk = nc.dram_tensor("bk", [dh], F32, kind="ExternalInput")
    bv = nc.dram_tensor("bv", [dh], F32, kind="ExternalInput")
    bo = nc.dram_tensor("bo", [dm], F32, kind="ExternalInput")
    m_tiles = nc.dram_tensor("m_tiles", [max(n_gen, 1), KB, QC], mybir.dt.int32,
                             kind="ExternalInput")
    y_out = nc.dram_tensor("y_out", [out_rows, dm], F32, kind="ExternalOutput")

    y_part = nc.dram_tensor("y_part", [s, dm], F32)
    y_rs = nc.dram_tensor("y_rs", [out_rows, dm], F32)

    with tile.TileContext(nc) as tc, ExitStack() as ex:
        persist = ex.enter_context(tc.tile_pool(name="persist", bufs=1))
        work = ex.enter_context(tc.tile_pool(name="work", bufs=3))
        ps_w = ex.enter_context(tc.tile_pool(name="ps_w", bufs=2, space="PSUM"))
        ps_s = ex.enter_context(tc.tile_pool(name="ps_s", bufs=2, space="PSUM"))
        ps_o = ex.enter_context(tc.tile_pool(name="ps_o", bufs=2, space="PSUM"))
        wscope = ExitStack()
        wpool = wscope.enter_context(tc.tile_pool(name="wpool", bufs=1))

        # ---- constants ----
        ident = persist.tile([P, P], st, tag="ident", name="ident")
        make_identity(nc, ident[:])
        # tri[k, q] = 1.0 where k <= q (keep), else 0
        tri = persist.tile([P, P], st, tag="tri", name="tri")
        nc.gpsimd.memset(tri[:], 0.0)
        nc.gpsimd.affine_select(out=tri[:], in_=tri[:],
                                compare_op=mybir.AluOpType.is_gt,
                                fill=1.0, base=0,
                                pattern=[[-1, P]], channel_multiplier=1)
        gen_sb = None
        if n_gen:
            gen_sb = persist.tile([P, n_gen, QC], st, tag="gen", name="gen")
            gi = persist.tile([P, n_gen, QC], mybir.dt.int32, tag="gen_i", name="gen_i")
            nc.sync.dma_start(gi[:], m_tiles[:].rearrange("n p q -> p n q"))
            for i in range(n_gen):
                nc.vector.tensor_copy(gen_sb[:, i, :], gi[:, i, :])

        # ---- load + cast weights (one batched DMA per tensor) ----
        dmac = nc.gpsimd.dma_start if st == BF16 else nc.sync.dma_start
        w_inT_b = wpool.tile([P, nde, dm], st, tag="w_inT", name="w_inT_b")
        hd = nde // 2
        dmac(out=w_inT_b[:, 0:hd, :],
             in_=w_inT[0:hd * P, :].rearrange("(u p) m -> p u m", p=P))
        dmac(out=w_inT_b[:, hd:nde, :],
             in_=w_inT[hd * P:, :].rearrange("(u p) m -> p u m", p=P))
        w_inT_sb = [w_inT_b[:, u, :] for u in range(nde)]
        w_sb = {}
        for name, w in (("q", wq), ("k", wk), ("v", wv)):
            wb = wpool.tile([P, nde, dh], st, tag=f"w{name}", name=f"w{name}_b")
            dmac(out=wb[:], in_=w[:].rearrange("(u p) d -> p u d", p=P))
            w_sb[name] = [wb[:, u, :] for u in range(nde)]
        wo_b = persist.tile([P, ndh, dm], st, tag="wo", name="wo_b")
        dmac(out=wo_b[:], in_=wo[:].rearrange("(t p) m -> p t m", p=P))
        wo_sb = [wo_b[:, t, :] for t in range(ndh)]
        b_inT = wpool.tile([P, nde], st, tag="b_inT", name="b_inT")
        dmac(out=b_inT[:], in_=b_in[:].rearrange("(t p) -> p t", p=P))

        # ---- combine weights: Wc_x = W_in @ Wx (+ bias fold) ----
        wc = {}
        bc = {}
        for name in ("q", "k", "v"):
            wc[name] = [persist.tile([P, dh], st, tag=f"wc{name}{t}", name=f"wc{name}{t}")
                        for t in range(ndm)]
            for t in range(ndm):
                ps = ps_w.tile([P, dh], F32, tag="ps_w", name="ps_w")
                for u in range(nde):
                    nc.tensor.matmul(
                        ps[:], mmv(w_inT_sb[u][:, t * P:(t + 1) * P]),
                        mmv(w_sb[name][u][:]),
                        start=(u == 0), stop=(u == nde - 1))
                nc.vector.tensor_copy(wc[name][t][:], ps[:])
            # bias: bc = Wx^T @ b_in + bx   -> [128, ndh] column layout
            bvec = {"q": bq, "k": bk, "v": bv}[name]
            bxT = wpool.tile([P, ndh], F32, tag=f"bxT{name}", name=f"bxT{name}")
            nc.sync.dma_start(out=bxT[:], in_=bvec[:].rearrange("(t p) -> p t", p=P))
            bc[name] = persist.tile([P, ndh], F32, tag=f"bc{name}", name=f"bc{name}")
            for t in range(ndh):
                ps = ps_w.tile([P, 1], F32, tag="ps_w", name="ps_w")
                for u in range(nde):
                    nc.tensor.matmul(
                        ps[:], mmv(w_sb[name][u][:, t * P:(t + 1) * P]),
                        mmv(b_inT[:, u:u + 1]),
                        start=(u == 0), stop=(u == nde - 1))
                nc.vector.tensor_add(bc[name][:, t:t + 1], ps[:], bxT[:, t:t + 1])

        wscope.close()   # frees weight-staging SBUF for the streaming pools
        xpool = ex.enter_context(tc.tile_pool(name="xpool", bufs=2))
        ppool = ex.enter_context(tc.tile_pool(name="ppool", bufs=4))
        ypool = ex.enter_context(tc.tile_pool(name="ypool", bufs=3))

        # ---- projections (transposed): X^T[dout, seq] per q-chunk ----
        qT_sb = [persist.tile([P, s], st, tag=f"qT{t}", name=f"qT{t}") for t in range(ndh)]
        kT_sb = [persist.tile([P, s], st, tag=f"kT{t}", name=f"kT{t}") for t in range(ndh)]
        v_aug = [persist.tile([P, nkb, DK + 1], st, tag=f"vaug{h}", name=f"vaug{h}")
                 for h in range(hloc)]
        for h in range(hloc):
            nc.gpsimd.memset(v_aug[h][:, :, DK], 1.0)

        def project_chunk(name, xdram, dst, iqc):
                xb = xpool.tile([P, ndm, QC], st, tag="xb", name="xb")
                dmac(out=xb[:],
                     in_=xdram[:, iqc * QC:(iqc + 1) * QC]
                         .rearrange("(u p) s -> p u s", p=P))
                xs = [xb[:, u, :] for u in range(ndm)]
                for t in range(ndh):
                    ps = ps_w.tile([P, QC], F32, tag="ps_w", name="ps_w")
                    for u in range(ndm):
                        nc.tensor.matmul(
                            ps[:], mmv(wc[name][u][:, t * P:(t + 1) * P]),
                            mmv(xs[u][:]), start=(u == 0), stop=(u == ndm - 1))
                    if dst is not None:
                        nc.vector.tensor_scalar_add(
                            dst[t][:, iqc * QC:(iqc + 1) * QC], ps[:],
                            bc[name][:, t:t + 1])
                    else:
                        # V: add bias, then PE-transpose into V_aug natural
                        vt = work.tile([P, QC], st, tag="vt", name="vt")
                        nc.vector.tensor_scalar_add(vt[:], ps[:],
                                                    bc[name][:, t:t + 1])
                        for hh in range(P // DK):      # heads in this chunk
                            h = t * (P // DK) + hh
                            for j in range(QC // P):   # k-blocks in chunk
                                ikb = iqc * (QC // P) + j
                                pst = ps_w.tile([P, DK], st, tag="ps_w", name="ps_w")
                                o = hh * DK
                                nc.tensor.transpose(
                                    pst[:],
                                    vt[o:o + DK, j * P:(j + 1) * P],
                                    ident[o:o + DK, o:o + DK])
                                nc.vector.tensor_copy(v_aug[h][:, ikb, 0:DK],
                                                      pst[:])

        # ---- attention ----
        GW = 2                      # kb blocks per psum_s tile (2 banks)
        cu = [persist.tile([P, s], st, tag=f"cu{t}", name=f"cu{t}") for t in range(ndh)]
        inv_sqrt = 1.0 / math.sqrt(DK)

        def attention_chunk(iqc):
            blist = blocks[iqc]
            for h in range(hloc):
                t, off = h // 2, (h % 2) * DK
                qv = qT_sb[t][off:off + DK, iqc * QC:(iqc + 1) * QC]
                po = ps_o.tile([DK + 1, QC], F32, tag="ps_av", name="ps_av")
                first = True
                for g0 in range(0, len(blist), GW):
                    grp = blist[g0:g0 + GW]
                    pss = ps_s.tile([P, GW * QC], F32, tag="ps_scores", name="ps_scores")
                    for j, (ikb, mode, arg) in enumerate(grp):
                        kv = kT_sb[t][off:off + DK, ikb * KB:(ikb + 1) * KB]
                        nc.tensor.matmul(pss[:, j * QC:(j + 1) * QC],
                                         mmv(kv), mmv(qv))
                    pt = ppool.tile([P, GW * QC], st, tag="p", name="p")
                    nw = len(grp) * QC
                    nc.scalar.activation(pt[:, 0:nw], pss[:, 0:nw],
                                         mybir.ActivationFunctionType.Exp,
                                         scale=inv_sqrt)
                    for j, (ikb, mode, arg) in enumerate(grp):
                        pj = pt[:, j * QC:(j + 1) * QC]
                        if mode == TRI:
                            r = arg
                            if r > 0:
                                nc.gpsimd.memset(pj[:, 0:r * P], 0.0)
                            nc.vector.tensor_mul(
                                pj[:, r * P:(r + 1) * P],
                                pj[:, r * P:(r + 1) * P], tri[:])
                        elif mode == GEN:
                            nc.vector.tensor_mul(pj[:], pj[:],
                                                 gen_sb[:, arg, :])
                    for j, (ikb, mode, arg) in enumerate(grp):
                        nc.tensor.matmul(
                            po[:], mmv(v_aug[h][:, ikb, :]),
                            mmv(pt[:, j * QC:(j + 1) * QC]),
                            start=first,
                            stop=(g0 + GW >= len(blist) and j == len(grp) - 1))
                        first = False
                rec1 = work.tile([1, QC], F32, tag="rec1", name="rec1")
                nc.vector.reciprocal(rec1[:], po[DK:DK + 1, :])
                recb = work.tile([DK, QC], F32, tag="recb", name="recb")
                nc.gpsimd.partition_broadcast(recb[:], rec1[:])
                # fused copy + normalize: cu = out_unnorm / denom
                nc.vector.tensor_mul(
                    cu[t][off:off + DK, iqc * QC:(iqc + 1) * QC],
                    po[0:DK, :], recb[:])

        # ---- output projection ----
        def yproj_block(qb):
            ys = ypool.tile([P, dm], F32, tag="y", name="y")
            for mb in range(dm // QC):
                ps = ps_w.tile([P, QC], F32, tag="ps_w", name="ps_w")
                for t in range(ndh):
                    nc.tensor.matmul(
                        ps[:], mmv(cu[t][:, qb * P:(qb + 1) * P]),
                        mmv(wo_sb[t][:, mb * QC:(mb + 1) * QC]),
                        start=(t == 0), stop=(t == ndh - 1))
                nc.vector.tensor_copy(ys[:, mb * QC:(mb + 1) * QC], ps[:])
            nc.sync.dma_start(out=y_part[qb * P:(qb + 1) * P, :], in_=ys[:])

        # per-chunk reduce-scatter, overlapped with later chunks' compute.
        # chunk c reduces y_part rows [c*QC, (c+1)*QC); rank r of the group
        # receives rows c*QC + [r*P, (r+1)*P) -> y_out rows [c*P, (c+1)*P).
        # the host reassembles the strided shards (see assemble()).
        def rs_chunk(c):
            if collective:
                nc.gpsimd.collective_compute(
                    "ReduceScatter", mybir.AluOpType.add,
                    replica_groups=GROUPS,
                    ins=[y_part[c * QC:(c + 1) * QC, :].opt()],
                    outs=[y_rs[c * P:(c + 1) * P, :].opt()])
            else:
                nc.sync.dma_start(out=y_rs[c * P:(c + 1) * P, :],
                                  in_=y_part[c * QC:c * QC + P, :])
            yo = ypool.tile([P, dm], F32, tag="y", name="yo")
            nc.sync.dma_start(out=yo[:],
                              in_=bo[:].unsqueeze(0).broadcast_to([P, dm]))
            nc.gpsimd.dma_start(out=yo[:], in_=y_rs[c * P:(c + 1) * P, :],
                                accum_op=mybir.AluOpType.add)
            nc.sync.dma_start(out=y_out[c * P:(c + 1) * P, :], in_=yo[:])

        # phase order: all projections, then attention (per chunk, with the
        # output projection and that chunk's reduce-scatter following early)
        for iqc in range(nqc):
            project_chunk("q", qT, qT_sb, iqc)
            project_chunk("k", kT, kT_sb, iqc)
            project_chunk("v", vT, None, iqc)
        for iqc in range(nqc):
            attention_chunk(iqc)
            for qb in range(iqc * (QC // P), (iqc + 1) * (QC // P)):
                yproj_block(qb)
            rs_chunk(iqc)

    nc.compile()
    return nc


# ------------------------------------------------------------------
_CACHE = {}


def _get_compiled(plan_key, blocks, n_gen, mm):
    if plan_key not in _CACHE:
        nc = build_mha(blocks, n_gen, mm=mm)
        nc.m = get_hw_module(nc.m)
        _CACHE[plan_key] = nc
    return _CACHE[plan_key]


def make_in_maps(q, k, v, mask, W_in, b_in, Wq, bq, Wk, bk, Wv, bv, Wo, bo,
                 blocks=None, n_gen=None, gen_tiles=None):
    if blocks is None:
        blocks, n_gen, gen_tiles = make_plan(mask)
    dh = DE // 4
    t = lambda a: np.ascontiguousarray(np.asarray(a).T)
    in_maps = []
    for c in range(N_CORES):
        b, g = c // 4, c % 4
        sl = slice(g * dh, (g + 1) * dh)
        mt = (gen_tiles[b] if n_gen else
              np.zeros((1, KB, QC), np.int32))
        in_maps.append({
            "qT": t(q[b]), "kT": t(k[b]), "vT": t(v[b]),
            "w_inT": t(W_in),
            "wq": np.ascontiguousarray(Wq[:, sl]),
            "wk": np.ascontiguousarray(Wk[:, sl]),
            "wv": np.ascontiguousarray(Wv[:, sl]),
            "wo": np.ascontiguousarray(Wo[sl, :]),
            "b_in": np.asarray(b_in),
            "bq": np.ascontiguousarray(bq[sl]),
            "bk": np.ascontiguousarray(bk[sl]),
            "bv": np.ascontiguousarray(bv[sl]),
            "bo": np.asarray(bo),
            "m_tiles": mt,
        })
    return in_maps, blocks, n_gen


def assemble(results):
    out = np.empty((B, S, DM), np.float32)
    for core in range(N_CORES):
        b, r = core // 4, core % 4
        y = results[core]["y_out"]            # [nqc*P, DM] strided shards
        for c in range(S // QC):
            out[b, c * QC + r * P:c * QC + (r + 1) * P, :] = \
                y[c * P:(c + 1) * P, :]
    return out


MM_MODE = "bf16"


def kernel(**inputs):
    mask = inputs["mask"]
    blocks, n_gen, gen_tiles = make_plan(np.asarray(mask))
    plan_key = (str(blocks), n_gen, MM_MODE)
    nc = _get_compiled(plan_key, blocks, n_gen, MM_MODE)
    in_maps, _, _ = make_in_maps(
        inputs["q"], inputs["k"], inputs["v"], mask,
        inputs["W_in"], inputs["b_in"], inputs["Wq"], inputs["bq"],
        inputs["Wk"], inputs["bk"], inputs["Wv"], inputs["bv"],
        inputs["Wo"], inputs["bo"],
        blocks=blocks, n_gen=n_gen, gen_tiles=gen_tiles)
    res = bass_utils.run_bass_kernel_spmd(nc, in_maps,
                                          core_ids=list(range(N_CORES)))
    return assemble(res.results)



# revision 7
# speedup vs baseline: 2.2841x; 2.2841x over previous
"""Trainium2 Bass kernel for nn_MultiHeadAttention_72069551227273.

Reference computation (B=2, S=2048, D_MODEL=D_EMB=1024, H=16, d_k=64):
    q_p = q @ W_in + b_in                    (shared input projection)
    qh  = heads(q_p @ Wq + bq)               (per-head projections)
    s   = qh @ kh^T / sqrt(d_k), causal-masked softmax
    out = (attn @ vh, concat heads) @ Wo + bo

Sharding: 8 cores = 2 (batch) x 4 (head groups of 4 heads / 256 emb cols).
Per core the input and head projections are fused algebraically:
    Q = q @ (W_in @ Wq_slice) + (b_in @ Wq_slice + bq_slice)
with the weight combine computed on device.  All activations that feed
matmuls are bf16; the host pre-casts inputs/weights to bf16 so every load
is a plain HWDGE DMA (no software-DGE cast descriptors).  Q/K live in
SBUF transposed ([feature, seq]); V is produced directly in natural
[seq, feature] orientation (stationary = x^T tile), so no PE transposes
are needed anywhere.  Softmax skips max-subtraction (scores are O(1))
and gets its denominator from a ones-column appended to V.  Mask
structure is resolved at trace time (skip / full / triangular / general
blocks); score blocks for a pair of heads are computed by two row-tiled
matmuls (64-row contraction each) sharing one PSUM tile and one Exp
activation instruction.

Collective: instead of reduce-scattering the f32 output partials
(8.4 MB/core), each core ships its bf16 transposed concat slice
[256, 512] per q-chunk through a single 8-rank AllToAll (256 KB/rank):
global rank r receives, for its own 64-row q-block of every chunk, the
full 1024-dim concat rows of BOTH batches.  The output projection then
runs fully local (contraction over all 1024 emb dims, the two batches
col-tiled into one PSUM tile), and per-chunk collectives overlap the
remaining attention compute.
"""

import sys

sys.path.append("/opt/trn_rl_repo")

import math
from contextlib import ExitStack

import numpy as np

import concourse.bass as bass
import concourse.bacc as bacc
import concourse.mybir as mybir
import concourse.tile as tile
from concourse import bass_utils
from concourse.bass_interp import get_hw_module

# problem dims
B, S, DM, DE, H, DK = 2, 2048, 1024, 1024, 16, 64
N_CORES = 8
P = 128                      # partitions
QC = 512                     # q chunk (psum bank width in fp32)
KB = 128                     # k block (scores^T partition block)
NQC, NKB, NDM, NDE = S // QC, S // KB, DM // P, DE // P
DH = DE // 4                 # per-core emb slice (256)
NDH = DH // P                # 2
HLOC = DH // DK              # heads per core (4)
QB = QC // N_CORES           # per-rank q rows per chunk (64)

F32 = mybir.dt.float32
BF16 = mybir.dt.bfloat16

FULL, TRI, GEN, SKIP = 0, 1, 2, 3


def make_plan(mask_np, s=S, qc=QC, kb=KB):
    """Classify scores^T blocks [kb x qc] from the (B, S, S) 0/1 mask.

    Returns (blocks, n_gen_tiles, gen_tiles_per_batch):
      blocks[iqc] = list of (ikb, mode, arg)
    """
    nqc, nkb = s // qc, s // kb
    m = np.asarray(mask_np) != 0          # [B, S(q), S(k)] True = attend
    tril = np.tril(np.ones((s, s), bool))
    causal = all(np.array_equal(m[b], tril) for b in range(m.shape[0]))
    blocks = []
    if causal:
        for iqc in range(nqc):
            row = []
            for ikb in range(nkb):
                if (ikb + 1) * kb <= iqc * qc:
                    row.append((ikb, FULL, 0))
                elif ikb * kb < (iqc + 1) * qc:
                    row.append((ikb, TRI, (ikb * kb - iqc * qc) // kb))
                # else fully masked -> skip
            blocks.append(row)
        return blocks, 0, None

    # general path: per-block classification, unioned across batches
    nb = m.shape[0]
    # every query row must attend to >= 1 key (else softmax semantics differ)
    assert m.any(axis=-1).all(), "fully-masked query rows unsupported"
    gen_tiles = [[] for _ in range(nb)]
    for iqc in range(nqc):
        row = []
        for ikb in range(nkb):
            sub = m[:, iqc * qc:(iqc + 1) * qc, ikb * kb:(ikb + 1) * kb]
            if sub.all():
                row.append((ikb, FULL, 0))
            elif not sub.any():
                continue
            else:
                idx = len(gen_tiles[0])
                for b in range(nb):
                    gen_tiles[b].append(sub[b].T.astype(np.int32))  # [kb, qc]
                row.append((ikb, GEN, idx))
        blocks.append(row)
    n_gen = len(gen_tiles[0])
    gt = [np.stack(g) if n_gen else np.zeros((1, kb, qc), np.int32)
          for g in gen_tiles]
    return blocks, n_gen, gt


def build_mha(blocks, n_gen, *, collective=True):
    """Trace the per-core MHA program."""
    st = BF16
    inv_sqrt = 1.0 / math.sqrt(DK)

    nc = bacc.Bacc("TRN2", target_bir_lowering=False, debug=False,
                   num_devices=N_CORES)

    # ---- kernel I/O (per core; bf16 pre-cast on host) ----
    qT = nc.dram_tensor("qT", [DM, S], st, kind="ExternalInput")
    kT = nc.dram_tensor("kT", [DM, S], st, kind="ExternalInput")
    vT = nc.dram_tensor("vT", [DM, S], st, kind="ExternalInput")
    w_inT = nc.dram_tensor("w_inT", [DE, DM], st, kind="ExternalInput")
    wq = nc.dram_tensor("wq", [DE, DH], st, kind="ExternalInput")
    wk = nc.dram_tensor("wk", [DE, DH], st, kind="ExternalInput")
    wv = nc.dram_tensor("wv", [DE, DH], st, kind="ExternalInput")
    wo = nc.dram_tensor("wo", [DE, DM], st, kind="ExternalInput")   # full Wo
    b_in = nc.dram_tensor("b_in", [DE], st, kind="ExternalInput")
    bq = nc.dram_tensor("bq", [DH], st, kind="ExternalInput")
    bk = nc.dram_tensor("bk", [DH], st, kind="ExternalInput")
    bv = nc.dram_tensor("bv", [DH], st, kind="ExternalInput")
    bo = nc.dram_tensor("bo", [DM], F32, kind="ExternalInput")
    m_tiles = nc.dram_tensor("m_tiles", [max(n_gen, 1), KB, QC],
                             mybir.dt.int32, kind="ExternalInput")
    # rows (chunk, batch, 64): chunk c gives q rows c*512 + rank*64 of both b
    y_out = nc.dram_tensor("y_out", [NQC * P, DM], F32, kind="ExternalOutput")

    # per-chunk AllToAll buffers: shard j = my concatT slice [256, 64] for
    # global rank j's q-block.  After A2A, slot i = rank i's dh slice for MY
    # q-block; slots 0-3 stack batch-0's 1024 concat dims, 4-7 batch-1's.
    cc_in = [nc.dram_tensor(f"cc_in{c}", [N_CORES, DH, QB], st)
             for c in range(NQC)]
    cc_out = [nc.dram_tensor(f"cc_out{c}", [N_CORES, DH, QB], st)
              for c in range(NQC)]

    with tile.TileContext(nc) as tc, ExitStack() as ex:
        persist = ex.enter_context(tc.tile_pool(name="persist", bufs=1))
        work = ex.enter_context(tc.tile_pool(name="work", bufs=3))
        ps_s = ex.enter_context(tc.tile_pool(name="ps_s", bufs=2, space="PSUM"))
        ps_o = ex.enter_context(tc.tile_pool(name="ps_o", bufs=1, space="PSUM"))
        ps_p = ex.enter_context(tc.tile_pool(name="ps_p", bufs=2, space="PSUM"))
        wscope = ExitStack()
        wpool = wscope.enter_context(tc.tile_pool(name="wpool", bufs=1))

        # ---- constants ----
        # tri2[k, j, q] = 1.0 where k <= q (keep), else 0; j = head-in-pair
        tri2 = persist.tile([P, 2, P], st, tag="tri2", name="tri2")
        nc.gpsimd.memset(tri2[:], 0.0)
        for j in range(2):
            nc.gpsimd.affine_select(out=tri2[:, j, :], in_=tri2[:, j, :],
                                    compare_op=mybir.AluOpType.is_gt,
                                    fill=1.0, base=0,
                                    pattern=[[-1, P]], channel_multiplier=1)
        ones1 = persist.tile([1, P], st, tag="ones1", name="ones1")
        nc.gpsimd.memset(ones1[:], 1.0)
        gen_sb = None
        if n_gen:
            gen_sb = persist.tile([P, n_gen, QC], st, tag="gen", name="gen")
            gi = persist.tile([P, n_gen, QC], mybir.dt.int32, tag="gen_i",
                              name="gen_i")
            nc.sync.dma_start(gi[:], m_tiles[:].rearrange("n p q -> p n q"))
            for i in range(n_gen):
                nc.vector.tensor_copy(gen_sb[:, i, :], gi[:, i, :])

        # ---- load weights (plain HWDGE; everything already bf16) ----
        w_inT_b = wpool.tile([P, NDE, DM], st, tag="w_inT", name="w_inT_b")
        hd = NDE // 2
        nc.sync.dma_start(out=w_inT_b[:, 0:hd, :],
                          in_=w_inT[0:hd * P, :].rearrange("(u p) m -> p u m", p=P))
        nc.sync.dma_start(out=w_inT_b[:, hd:NDE, :],
                          in_=w_inT[hd * P:, :].rearrange("(u p) m -> p u m", p=P))
        w_inT_sb = [w_inT_b[:, u, :] for u in range(NDE)]
        w_sb = {}
        for name, w in (("q", wq), ("k", wk), ("v", wv)):
            wb = wpool.tile([P, NDE, DH], st, tag=f"w{name}", name=f"w{name}_b")
            nc.sync.dma_start(out=wb[:], in_=w[:].rearrange("(u p) d -> p u d", p=P))
            w_sb[name] = [wb[:, u, :] for u in range(NDE)]
        wo_b = persist.tile([P, NDE, DM], st, tag="wo", name="wo_b")
        nc.sync.dma_start(out=wo_b[:], in_=wo[:].rearrange("(u p) m -> p u m", p=P))
        wo_sb = [wo_b[:, u, :] for u in range(NDE)]
        b_inT = wpool.tile([P, NDE], st, tag="b_inT", name="b_inT")
        nc.sync.dma_start(out=b_inT[:], in_=b_in[:].rearrange("(t p) -> p t", p=P))
        bo_b = persist.tile([P, DM], F32, tag="bo_b", name="bo_b")
        nc.sync.dma_start(out=bo_b[:],
                          in_=bo[:].unsqueeze(0).broadcast_to([P, DM]))

        # ---- combine weights: Wc_x = W_in @ Wx (+ bias fold) ----
        wc = {}
        bc = {}
        for name in ("q", "k", "v"):
            wc[name] = [persist.tile([P, DH], st, tag=f"wc{name}{t}",
                                     name=f"wc{name}{t}") for t in range(NDM)]
            for t in range(NDM):
                ps = ps_p.tile([P, DH], F32, tag="ps_w", name="ps_w")
                for u in range(NDE):
                    nc.tensor.matmul(
                        ps[:], w_inT_sb[u][:, t * P:(t + 1) * P],
                        w_sb[name][u][:],
                        start=(u == 0), stop=(u == NDE - 1))
                nc.vector.tensor_copy(wc[name][t][:], ps[:])
        # Q/K bias (column layout, added per-partition after projection)
        for name, bvec in (("q", bq), ("k", bk)):
            bxT = wpool.tile([P, NDH], st, tag=f"bxT{name}", name=f"bxT{name}")
            nc.sync.dma_start(out=bxT[:], in_=bvec[:].rearrange("(t p) -> p t", p=P))
            bc[name] = persist.tile([P, NDH], F32, tag=f"bc{name}", name=f"bc{name}")
            for t in range(NDH):
                ps = ps_p.tile([P, 1], F32, tag="ps_w", name="ps_b")
                for u in range(NDE):
                    nc.tensor.matmul(
                        ps[:], w_sb[name][u][:, t * P:(t + 1) * P],
                        b_inT[:, u:u + 1],
                        start=(u == 0), stop=(u == NDE - 1))
                nc.vector.tensor_add(bc[name][:, t:t + 1], ps[:], bxT[:, t:t + 1])
        # V bias (row layout, accumulated into the psum by a ones matmul)
        bv_row = wpool.tile([1, DH], st, tag="bv_row", name="bv_row")
        nc.sync.dma_start(out=bv_row[:], in_=bv[:].unsqueeze(0))
        bcv = persist.tile([1, DH], st, tag="bcv", name="bcv")
        psb = ps_p.tile([1, DH], F32, tag="ps_w", name="ps_b")
        for u in range(NDE):
            nc.tensor.matmul(psb[:], b_inT[:, u:u + 1], w_sb["v"][u][:],
                             start=(u == 0), stop=(u == NDE - 1))
        nc.vector.tensor_add(bcv[:], psb[:], bv_row[:])

        wscope.close()   # frees weight-staging SBUF for the streaming pools
        xpool = ex.enter_context(tc.tile_pool(name="xpool", bufs=2))
        ppool = ex.enter_context(tc.tile_pool(name="ppool", bufs=3))
        cupool = ex.enter_context(tc.tile_pool(name="cupool", bufs=2))
        rbpool = ex.enter_context(tc.tile_pool(name="rbpool", bufs=2))
        ypool = ex.enter_context(tc.tile_pool(name="ypool", bufs=2))

        # ---- persistent activation tiles ----
        qT_sb = [persist.tile([P, S], st, tag=f"qT{t}", name=f"qT{t}")
                 for t in range(NDH)]
        kT_sb = [persist.tile([P, S], st, tag=f"kT{t}", name=f"kT{t}")
                 for t in range(NDH)]
        # V natural [key, dv] with a ones column per head: head h occupies
        # cols h*65 .. h*65+64 (65th col = softmax denominator ones)
        v_aug = persist.tile([P, NKB, HLOC * (DK + 1)], st, tag="vaug",
                             name="vaug")
        for h in range(HLOC):
            nc.gpsimd.memset(v_aug[:, :, h * (DK + 1) + DK:
                                    h * (DK + 1) + DK + 1], 1.0)

        # ---- input staging ----
        xtiles = {}

        def load_x(c):
            for name, xdram in (("q", qT), ("k", kT), ("v", vT)):
                xt = xpool.tile([P, NDM, QC], st, tag=f"x{name}",
                                name=f"x{name}{c}")
                nc.sync.dma_start(
                    out=xt[:],
                    in_=xdram[:, c * QC:(c + 1) * QC]
                        .rearrange("(u p) s -> p u s", p=P))
                xtiles[(name, c)] = xt

        # ---- projection units (Q/K transposed, V natural) ----
        def proj_qk_unit(name, c, t):
            xs = xtiles[(name, c)]
            dst = qT_sb if name == "q" else kT_sb
            ps = ps_p.tile([P, QC], F32, tag="ps_w", name="ps_w")
            for u in range(NDM):
                nc.tensor.matmul(
                    ps[:], wc[name][u][:, t * P:(t + 1) * P], xs[:, u, :],
                    start=(u == 0), stop=(u == NDM - 1))
            nc.vector.tensor_scalar_add(
                dst[t][:, c * QC:(c + 1) * QC], ps[:], bc[name][:, t:t + 1])

        def proj_v_unit(c, jb):
            xs = xtiles[("v", c)]
            ikb = c * (QC // P) + jb
            ps = ps_p.tile([P, DH], F32, tag="ps_w", name="ps_w")
            for u in range(NDM):
                nc.tensor.matmul(
                    ps[:], xs[:, u, jb * P:(jb + 1) * P], wc["v"][u][:],
                    start=(u == 0), stop=False)
            nc.tensor.matmul(ps[:], ones1[:], bcv[:], start=False, stop=True)
            nc.vector.tensor_copy(
                v_aug[:, ikb, :].rearrange("p (h u) -> p h u", h=HLOC)[:, :, 0:DK],
                ps[:].rearrange("p (h u) -> p h u", h=HLOC))

        def proj_units(c):
            return ([lambda t=t: proj_qk_unit("q", c, t) for t in range(NDH)]
                    + [lambda t=t: proj_qk_unit("k", c, t) for t in range(NDH)]
                    + [lambda j=j: proj_v_unit(c, j) for j in range(QC // P)])

        # ---- attention ----
        def attention_chunk(c, inject):
            """inject: iterator of closures run between block iterations."""
            blist = blocks[c]
            qsl = slice(c * QC, (c + 1) * QC)
            cu = cupool.tile([P, NDH, QC], st, tag="cu", name=f"cu{c}")
            for p in range(NDH):        # head pair p = heads (2p, 2p+1)
                po = [ps_o.tile([DK + 1, QC], F32, tag=f"po{j}", name=f"po{j}")
                      for j in range(2)]
                nblk = len(blist)
                for gi_, (ikb, mode, arg) in enumerate(blist):
                    ksl = slice(ikb * KB, (ikb + 1) * KB)
                    pss = ps_s.tile([P, 2, QC], F32, tag="pss", name="pss")
                    # two row-tiled 64-contraction matmuls fill both halves
                    nc.tensor.matmul(pss[:, 0, :], kT_sb[p][0:DK, ksl],
                                     qT_sb[p][0:DK, qsl])
                    nc.tensor.matmul(pss[:, 1, :], kT_sb[p][DK:P, ksl],
                                     qT_sb[p][DK:P, qsl])
                    pt = ppool.tile([P, 2, QC], st, tag="pt", name="pt")
                    nc.scalar.activation(pt[:], pss[:],
                                         mybir.ActivationFunctionType.Exp,
                                         scale=inv_sqrt)
                    if mode == TRI:
                        r = arg
                        if r > 0:
                            nc.vector.memset(pt[:, :, 0:r * P], 0.0)
                        nc.vector.tensor_mul(pt[:, :, r * P:(r + 1) * P],
                                             pt[:, :, r * P:(r + 1) * P],
                                             tri2[:])
                    elif mode == GEN:
                        for j in range(2):
                            nc.vector.tensor_mul(pt[:, j, :], pt[:, j, :],
                                                 gen_sb[:, arg, :])
                    for j in range(2):
                        h = 2 * p + j
                        nc.tensor.matmul(
                            po[j][:],
                            v_aug[:, ikb, h * (DK + 1):(h + 1) * (DK + 1)],
                            pt[:, j, :],
                            start=(gi_ == 0), stop=(gi_ == nblk - 1))
                    for f in inject.pop_some(pair=p, gi=gi_, nblk=nblk):
                        f()
                # normalize: cu rows j*64.. of tile column p
                for j in range(2):
                    rec1 = work.tile([1, QC], F32, tag="rec1", name="rec1")
                    nc.vector.reciprocal(rec1[:], po[j][DK:DK + 1, :])
                    recb = work.tile([DK, QC], F32, tag="recb", name="recb")
                    nc.gpsimd.partition_broadcast(recb[:], rec1[:])
                    nc.vector.tensor_mul(cu[j * DK:(j + 1) * DK, p, :],
                                         po[j][0:DK, :], recb[:])
            # ship concatT chunk: shard r = cols r*64..(r+1)*64
            for t in range(NDH):
                nc.sync.dma_start(
                    out=cc_in[c][:, t * P:(t + 1) * P, :]
                        .rearrange("j p q -> p j q"),
                    in_=cu[:, t, :].rearrange("p (j q) -> p j q", j=N_CORES))
            if collective:
                nc.gpsimd.collective_compute(
                    "AllToAll", mybir.AluOpType.bypass,
                    replica_groups=[list(range(N_CORES))],
                    ins=[cc_in[c][:].opt()],
                    outs=[cc_out[c][:].opt()])
            else:
                nc.sync.dma_start(out=cc_out[c][:], in_=cc_in[c][:])

        class Injector:
            """Spreads a chunk's projection units across attention blocks."""

            def __init__(self, units):
                self.units = list(units)
                self.total = max(1, len(units))
                self.emitted = 0
                self.seen = 0

            def pop_some(self, pair, gi, nblk):
                total_slots = 2 * nblk
                self.seen += 1
                want = (self.seen * self.total + total_slots - 1) // total_slots
                out = []
                while self.units and self.emitted < want:
                    out.append(self.units.pop(0))
                    self.emitted += 1
                return out

        # ---- output projection (after AllToAll of chunk c) ----
        def outproj_chunk(c):
            # rb[p, t, i, q]: slot i = rank i's dh slice (i<4: batch 0 dims,
            # i>=4: batch 1), t = 128-row half of that slice
            rb = rbpool.tile([P, NDH, N_CORES, QB], st, tag="rb",
                             name=f"rb{c}")
            for t in range(NDH):
                nc.sync.dma_start(
                    out=rb[:, t, :, :],
                    in_=cc_out[c][:, t * P:(t + 1) * P, :]
                        .rearrange("i p q -> p i q"))
            ys = ypool.tile([P, DM], F32, tag="ys", name=f"ys{c}")
            for half in range(DM // QC):
                y2 = ps_p.tile([P, QC], F32, tag="ps_w", name="ps_y")
                msl = slice(half * QC, (half + 1) * QC)
                for u in range(NDE):
                    # batch 0 -> psum rows 0:64, batch 1 -> rows 64:128
                    # (col-tiled pair, auto tile_position from base partition)
                    nc.tensor.matmul(y2[0:QB, :], rb[:, u % 2, u // 2, :],
                                     wo_sb[u][:, msl],
                                     start=(u == 0), stop=(u == NDE - 1))
                    nc.tensor.matmul(y2[QB:2 * QB, :],
                                     rb[:, u % 2, 4 + u // 2, :],
                                     wo_sb[u][:, msl],
                                     start=(u == 0), stop=(u == NDE - 1))
                nc.vector.tensor_add(ys[:, msl], y2[:], bo_b[:, msl])
            nc.sync.dma_start(out=y_out[c * P:(c + 1) * P, :], in_=ys[:])

        # ---- phase schedule ----
        load_x(0)
        for f in proj_units(0):
            f()
        for c in range(NQC):
            if c + 1 < NQC:
                load_x(c + 1)
                inj = Injector(proj_units(c + 1))
            else:
                inj = Injector([])
            attention_chunk(c, inj)
        for c in range(NQC):
            outproj_chunk(c)

    nc.compile()
    return nc


# ------------------------------------------------------------------
_CACHE = {}


def _get_compiled(plan_key, blocks, n_gen):
    if plan_key not in _CACHE:
        nc = build_mha(blocks, n_gen)
        nc.m = get_hw_module(nc.m)
        _CACHE[plan_key] = nc
    return _CACHE[plan_key]


def make_in_maps(q, k, v, mask, W_in, b_in, Wq, bq, Wk, bk, Wv, bv, Wo, bo,
                 blocks=None, n_gen=None, gen_tiles=None):
    import ml_dtypes
    bf = ml_dtypes.bfloat16
    if blocks is None:
        blocks, n_gen, gen_tiles = make_plan(mask)
    tb = lambda a: np.ascontiguousarray(np.asarray(a).T).astype(bf)
    cb = lambda a: np.ascontiguousarray(np.asarray(a)).astype(bf)
    in_maps = []
    for c in range(N_CORES):
        b, g = c // 4, c % 4
        sl = slice(g * DH, (g + 1) * DH)
        mt = (gen_tiles[b] if n_gen else
              np.zeros((1, KB, QC), np.int32))
        in_maps.append({
            "qT": tb(q[b]), "kT": tb(k[b]), "vT": tb(v[b]),
            "w_inT": tb(W_in),
            "wq": cb(Wq[:, sl]),
            "wk": cb(Wk[:, sl]),
            "wv": cb(Wv[:, sl]),
            "wo": cb(Wo),
            "b_in": cb(b_in),
            "bq": cb(bq[sl]),
            "bk": cb(bk[sl]),
            "bv": cb(bv[sl]),
            "bo": np.asarray(bo, np.float32),
            "m_tiles": mt,
        })
    return in_maps, blocks, n_gen


def assemble(results):
    out = np.empty((B, S, DM), np.float32)
    for r in range(N_CORES):
        y = results[r]["y_out"]               # [NQC*128, DM]
        for c in range(NQC):
            rows = slice(c * QC + r * QB, c * QC + (r + 1) * QB)
            out[0, rows, :] = y[c * P:c * P + QB, :]
            out[1, rows, :] = y[c * P + QB:(c + 1) * P, :]
    return out


def kernel(**inputs):
    mask = inputs["mask"]
    blocks, n_gen, gen_tiles = make_plan(np.asarray(mask))
    plan_key = (str(blocks), n_gen)
    nc = _get_compiled(plan_key, blocks, n_gen)
    in_maps, _, _ = make_in_maps(
        inputs["q"], inputs["k"], inputs["v"], mask,
        inputs["W_in"], inputs["b_in"], inputs["Wq"], inputs["bq"],
        inputs["Wk"], inputs["bk"], inputs["Wv"], inputs["bv"],
        inputs["Wo"], inputs["bo"],
        blocks=blocks, n_gen=n_gen, gen_tiles=gen_tiles)
    res = bass_utils.run_bass_kernel_spmd(nc, in_maps,
                                          core_ids=list(range(N_CORES)))
    return assemble(res.results)


# revision 16
# speedup vs baseline: 2.8443x; 1.2453x over previous
"""Trainium2 Bass kernel for nn_MultiHeadAttention_72069551227273.

Reference computation (B=2, S=2048, D_MODEL=D_EMB=1024, H=16, d_k=64):
    q_p = q @ W_in + b_in                    (shared input projection)
    qh  = heads(q_p @ Wq + bq)               (per-head projections)
    s   = qh @ kh^T / sqrt(d_k), causal-masked softmax
    out = (attn @ vh, concat heads) @ Wo + bo

Sharding: 8 cores = 2 (batch) x 4 (head groups of 4 heads / 256 emb cols).
Per core the input and head projections are fused algebraically:
    Q = q @ (W_in @ Wq_slice) + (b_in @ Wq_slice + bq_slice)
with the weight combine computed on device.  All activations that feed
matmuls are bf16; the host pre-casts inputs/weights to bf16 so every load
is a plain HWDGE DMA (no software-DGE cast descriptors).  Q/K live in
SBUF transposed ([feature, seq]); V is produced directly in natural
[seq, feature] orientation (stationary = x^T tile), so no PE transposes
are needed anywhere.  Softmax skips max-subtraction (scores are O(1))
and gets its denominator from a ones-column appended to V.  Mask
structure is resolved at trace time (skip / full / triangular / general
blocks); score blocks for a pair of heads are computed by two row-tiled
matmuls (64-row contraction each) sharing one PSUM tile and one Exp
activation instruction.

Collective: instead of reduce-scattering the f32 output partials
(8.4 MB/core), each core ships its bf16 transposed concat slice
[256, 512] per q-chunk through a single 8-rank AllToAll (256 KB/rank):
global rank r receives, for its own 64-row q-block of every chunk, the
full 1024-dim concat rows of BOTH batches.  The output projection then
runs fully local (contraction over all 1024 emb dims, the two batches
col-tiled into one PSUM tile), and per-chunk collectives overlap the
remaining attention compute.
"""

import sys

sys.path.append("/opt/trn_rl_repo")

import math
from contextlib import ExitStack

import numpy as np

import concourse.bass as bass
import concourse.bacc as bacc
import concourse.mybir as mybir
import concourse.tile as tile
from concourse import bass_utils
from concourse.bass_interp import get_hw_module

# problem dims
B, S, DM, DE, H, DK = 2, 2048, 1024, 1024, 16, 64
N_CORES = 8
P = 128                      # partitions
QC = 512                     # q chunk (psum bank width in fp32)
KB = 128                     # k block (scores^T partition block)
NQC, NKB, NDM, NDE = S // QC, S // KB, DM // P, DE // P
DH = DE // 4                 # per-core emb slice (256)
NDH = DH // P                # 2
HLOC = DH // DK              # heads per core (4)
QB = QC // N_CORES           # per-rank q rows per chunk (64)

F32 = mybir.dt.float32
BF16 = mybir.dt.bfloat16

FULL, TRI, GEN, SKIP = 0, 1, 2, 3


def make_plan(mask_np, s=S, qc=QC, kb=KB):
    """Classify scores^T blocks [kb x qc] from the (B, S, S) 0/1 mask.

    Returns (blocks, n_gen_tiles, gen_tiles_per_batch):
      blocks[iqc] = list of (ikb, mode, arg)
    """
    nqc, nkb = s // qc, s // kb
    m = np.asarray(mask_np) != 0          # [B, S(q), S(k)] True = attend
    tril = np.tril(np.ones((s, s), bool))
    causal = all(np.array_equal(m[b], tril) for b in range(m.shape[0]))
    blocks = []
    if causal:
        for iqc in range(nqc):
            row = []
            for ikb in range(nkb):
                if (ikb + 1) * kb <= iqc * qc:
                    row.append((ikb, FULL, 0))
                elif ikb * kb < (iqc + 1) * qc:
                    row.append((ikb, TRI, (ikb * kb - iqc * qc) // kb))
                # else fully masked -> skip
            blocks.append(row)
        return blocks, 0, None

    # general path: per-block classification, unioned across batches
    nb = m.shape[0]
    # every query row must attend to >= 1 key (else softmax semantics differ)
    assert m.any(axis=-1).all(), "fully-masked query rows unsupported"
    gen_tiles = [[] for _ in range(nb)]
    for iqc in range(nqc):
        row = []
        for ikb in range(nkb):
            sub = m[:, iqc * qc:(iqc + 1) * qc, ikb * kb:(ikb + 1) * kb]
            if sub.all():
                row.append((ikb, FULL, 0))
            elif not sub.any():
                continue
            else:
                idx = len(gen_tiles[0])
                for b in range(nb):
                    gen_tiles[b].append(sub[b].T.astype(np.int32))  # [kb, qc]
                row.append((ikb, GEN, idx))
        blocks.append(row)
    n_gen = len(gen_tiles[0])
    gt = [np.stack(g) if n_gen else np.zeros((1, kb, qc), np.int32)
          for g in gen_tiles]
    return blocks, n_gen, gt


def build_mha(blocks, n_gen, *, collective=True):
    """Trace the per-core MHA program."""
    st = BF16
    inv_sqrt = 1.0 / math.sqrt(DK)

    nc = bacc.Bacc("TRN2", target_bir_lowering=False, debug=False,
                   num_devices=N_CORES)

    # ---- kernel I/O (per core; bf16 pre-cast on host) ----
    qT = nc.dram_tensor("qT", [DM, S], st, kind="ExternalInput")
    kT = nc.dram_tensor("kT", [DM, S], st, kind="ExternalInput")
    vT = nc.dram_tensor("vT", [DM, S], st, kind="ExternalInput")
    w_inT = nc.dram_tensor("w_inT", [DE, DM], st, kind="ExternalInput")
    wq = nc.dram_tensor("wq", [DE, DH], st, kind="ExternalInput")
    wk = nc.dram_tensor("wk", [DE, DH], st, kind="ExternalInput")
    wv = nc.dram_tensor("wv", [DE, DH], st, kind="ExternalInput")
    wo = nc.dram_tensor("wo", [DE, DM], st, kind="ExternalInput")   # full Wo
    b_in = nc.dram_tensor("b_in", [DE], st, kind="ExternalInput")
    bq = nc.dram_tensor("bq", [DH], st, kind="ExternalInput")
    bk = nc.dram_tensor("bk", [DH], st, kind="ExternalInput")
    bv = nc.dram_tensor("bv", [DH], st, kind="ExternalInput")
    bo = nc.dram_tensor("bo", [DM], F32, kind="ExternalInput")
    m_tiles = nc.dram_tensor("m_tiles", [max(n_gen, 1), KB, QC],
                             mybir.dt.int32, kind="ExternalInput")
    # rows (chunk, batch, 64): chunk c gives q rows c*512 + rank*64 of both b
    y_out = nc.dram_tensor("y_out", [NQC * P, DM], F32, kind="ExternalOutput")

    # per-chunk AllToAll buffers: shard j = my concatT slice [256, 64] for
    # global rank j's q-block.  After A2A, slot i = rank i's dh slice for MY
    # q-block; slots 0-3 stack batch-0's 1024 concat dims, 4-7 batch-1's.
    cc_in = [nc.dram_tensor(f"cc_in{c}", [N_CORES, DH, QB], st)
             for c in range(NQC)]
    cc_out = [nc.dram_tensor(f"cc_out{c}", [N_CORES, DH, QB], st)
              for c in range(NQC)]

    with tile.TileContext(nc) as tc, ExitStack() as ex:
        persist = ex.enter_context(tc.tile_pool(name="persist", bufs=1))
        work = ex.enter_context(tc.tile_pool(name="work", bufs=3))
        ps_s = ex.enter_context(tc.tile_pool(name="ps_s", bufs=2, space="PSUM"))
        ps_o = ex.enter_context(tc.tile_pool(name="ps_o", bufs=1, space="PSUM"))
        ps_p = ex.enter_context(tc.tile_pool(name="ps_p", bufs=2, space="PSUM"))
        xpool = ex.enter_context(tc.tile_pool(name="xpool", bufs=2))
        wscope = ExitStack()
        wpool = wscope.enter_context(tc.tile_pool(name="wpool", bufs=1))

        # ---- constants ----
        # tri2[k, j, q] = 1.0 where k <= q (keep), else 0; j = head-in-pair
        tri2 = persist.tile([P, 2, P], st, tag="tri2", name="tri2")
        nc.gpsimd.memset(tri2[:], 0.0)
        for j in range(2):
            nc.gpsimd.affine_select(out=tri2[:, j, :], in_=tri2[:, j, :],
                                    compare_op=mybir.AluOpType.is_gt,
                                    fill=1.0, base=0,
                                    pattern=[[-1, P]], channel_multiplier=1)
        ones1 = persist.tile([1, P], st, tag="ones1", name="ones1")
        nc.gpsimd.memset(ones1[:], 1.0)
        gen_sb = None
        if n_gen:
            gen_sb = persist.tile([P, n_gen, QC], st, tag="gen", name="gen")
            gi = persist.tile([P, n_gen, QC], mybir.dt.int32, tag="gen_i",
                              name="gen_i")
            nc.sync.dma_start(gi[:], m_tiles[:].rearrange("n p q -> p n q"))
            for i in range(n_gen):
                nc.vector.tensor_copy(gen_sb[:, i, :], gi[:, i, :])

        # ---- load weights (plain HWDGE; everything already bf16) ----
        w_inT_b = wpool.tile([P, NDE, DM], st, tag="w_inT", name="w_inT_b")
        hd = NDE // 2
        nc.sync.dma_start(out=w_inT_b[:, 0:hd, :],
                          in_=w_inT[0:hd * P, :].rearrange("(u p) m -> p u m", p=P))
        nc.sync.dma_start(out=w_inT_b[:, hd:NDE, :],
                          in_=w_inT[hd * P:, :].rearrange("(u p) m -> p u m", p=P))
        w_inT_sb = [w_inT_b[:, u, :] for u in range(NDE)]
        w_sb = {}
        for name, w in (("q", wq), ("k", wk), ("v", wv)):
            wb = wpool.tile([P, NDE, DH], st, tag=f"w{name}", name=f"w{name}_b")
            nc.sync.dma_start(out=wb[:], in_=w[:].rearrange("(u p) d -> p u d", p=P))
            w_sb[name] = [wb[:, u, :] for u in range(NDE)]
        b_inT = wpool.tile([P, NDE], st, tag="b_inT", name="b_inT")
        nc.sync.dma_start(out=b_inT[:], in_=b_in[:].rearrange("(t p) -> p t", p=P))

        # ---- input staging (x(0) queued right behind the combine weights,
        # so it lands while the weight combine computes) ----
        xtiles = {}

        def load_x(c):
            for name, xdram in (("q", qT), ("k", kT), ("v", vT)):
                xt = xpool.tile([P, NDM, QC], st, tag=f"x{name}",
                                name=f"x{name}{c}")
                nc.sync.dma_start(
                    out=xt[:],
                    in_=xdram[:, c * QC:(c + 1) * QC]
                        .rearrange("(u p) s -> p u s", p=P))
                xtiles[(name, c)] = xt

        load_x(0)

        # ---- combine weights: Wc_x = W_in @ Wx (+ bias fold) ----
        wc = {}
        bc = {}
        for name in ("q", "k", "v"):
            wc[name] = [persist.tile([P, DH], st, tag=f"wc{name}{t}",
                                     name=f"wc{name}{t}") for t in range(NDM)]
            for t in range(NDM):
                ps = ps_p.tile([P, DH], F32, tag="ps_w", name="ps_w")
                for u in range(NDE):
                    nc.tensor.matmul(
                        ps[:], w_inT_sb[u][:, t * P:(t + 1) * P],
                        w_sb[name][u][:],
                        start=(u == 0), stop=(u == NDE - 1))
                nc.vector.tensor_copy(wc[name][t][:], ps[:])
        # Q/K bias (column layout, added per-partition after projection)
        for name, bvec in (("q", bq), ("k", bk)):
            bxT = wpool.tile([P, NDH], st, tag=f"bxT{name}", name=f"bxT{name}")
            nc.sync.dma_start(out=bxT[:], in_=bvec[:].rearrange("(t p) -> p t", p=P))
            bc[name] = persist.tile([P, NDH], F32, tag=f"bc{name}", name=f"bc{name}")
            for t in range(NDH):
                ps = ps_p.tile([P, 1], F32, tag="ps_w", name="ps_b")
                for u in range(NDE):
                    nc.tensor.matmul(
                        ps[:], w_sb[name][u][:, t * P:(t + 1) * P],
                        b_inT[:, u:u + 1],
                        start=(u == 0), stop=(u == NDE - 1))
                nc.vector.tensor_add(bc[name][:, t:t + 1], ps[:], bxT[:, t:t + 1])
        # V bias (row layout, accumulated into the psum by a ones matmul)
        bv_row = wpool.tile([1, DH], st, tag="bv_row", name="bv_row")
        nc.sync.dma_start(out=bv_row[:], in_=bv[:].unsqueeze(0))
        bcv = persist.tile([1, DH], st, tag="bcv", name="bcv")
        psb = ps_p.tile([1, DH], F32, tag="ps_w", name="ps_b")
        for u in range(NDE):
            nc.tensor.matmul(psb[:], b_inT[:, u:u + 1], w_sb["v"][u][:],
                             start=(u == 0), stop=(u == NDE - 1))
        nc.vector.tensor_add(bcv[:], psb[:], bv_row[:])

        # late-needed weights, emitted after the x(0) loads in queue order
        wo_b = persist.tile([P, NDE, DM], st, tag="wo", name="wo_b")
        nc.sync.dma_start(out=wo_b[:], in_=wo[:].rearrange("(u p) m -> p u m", p=P))
        wo_sb = [wo_b[:, u, :] for u in range(NDE)]
        bo_b = persist.tile([P, DM], F32, tag="bo_b", name="bo_b")
        nc.sync.dma_start(out=bo_b[:],
                          in_=bo[:].unsqueeze(0).broadcast_to([P, DM]))

        wscope.close()   # frees weight-staging SBUF for the streaming pools
        ppool = ex.enter_context(tc.tile_pool(name="ppool", bufs=3))
        cupool = ex.enter_context(tc.tile_pool(name="cupool", bufs=2))
        rbpool = ex.enter_context(tc.tile_pool(name="rbpool", bufs=2))
        ypool = ex.enter_context(tc.tile_pool(name="ypool", bufs=2))

        # ---- persistent activation tiles ----
        qT_sb = [persist.tile([P, S], st, tag=f"qT{t}", name=f"qT{t}")
                 for t in range(NDH)]
        kT_sb = [persist.tile([P, S], st, tag=f"kT{t}", name=f"kT{t}")
                 for t in range(NDH)]
        # V natural [key, dv] with a ones column per head: head h occupies
        # cols h*65 .. h*65+64 (65th col = softmax denominator ones)
        v_aug = persist.tile([P, NKB, HLOC * (DK + 1)], st, tag="vaug",
                             name="vaug")
        for h in range(HLOC):
            nc.gpsimd.memset(v_aug[:, :, h * (DK + 1) + DK:
                                    h * (DK + 1) + DK + 1], 1.0)

        # ---- projection units (Q/K transposed, V natural) ----
        def proj_qk_unit(name, c, t):
            xs = xtiles[(name, c)]
            dst = qT_sb if name == "q" else kT_sb
            ps = ps_p.tile([P, QC], F32, tag="ps_w", name="ps_w")
            for u in range(NDM):
                nc.tensor.matmul(
                    ps[:], wc[name][u][:, t * P:(t + 1) * P], xs[:, u, :],
                    start=(u == 0), stop=(u == NDM - 1))
            nc.vector.tensor_scalar_add(
                dst[t][:, c * QC:(c + 1) * QC], ps[:], bc[name][:, t:t + 1])

        def proj_v_unit(c, jb):
            xs = xtiles[("v", c)]
            ikb = c * (QC // P) + jb
            ps = ps_p.tile([P, DH], F32, tag="ps_w", name="ps_w")
            for u in range(NDM):
                nc.tensor.matmul(
                    ps[:], xs[:, u, jb * P:(jb + 1) * P], wc["v"][u][:],
                    start=(u == 0), stop=False)
            nc.tensor.matmul(ps[:], ones1[:], bcv[:], start=False, stop=True)
            nc.vector.tensor_copy(
                v_aug[:, ikb, :].rearrange("p (h u) -> p h u", h=HLOC)[:, :, 0:DK],
                ps[:].rearrange("p (h u) -> p h u", h=HLOC))

        def proj_units(c):
            return ([lambda t=t: proj_qk_unit("q", c, t) for t in range(NDH)]
                    + [lambda t=t: proj_qk_unit("k", c, t) for t in range(NDH)]
                    + [lambda j=j: proj_v_unit(c, j) for j in range(QC // P)])

        # ---- attention ----
        def attention_chunk(c, inject):
            """inject: iterator of closures run between block iterations."""
            blist = blocks[c]
            qsl = slice(c * QC, (c + 1) * QC)
            cu = cupool.tile([P, NDH, QC], st, tag="cu", name=f"cu{c}")
            for p in range(NDH):        # head pair p = heads (2p, 2p+1)
                po = [ps_o.tile([DK + 1, QC], F32, tag=f"po{j}", name=f"po{j}")
                      for j in range(2)]
                nblk = len(blist)
                for gi_, (ikb, mode, arg) in enumerate(blist):
                    ksl = slice(ikb * KB, (ikb + 1) * KB)
                    pss = ps_s.tile([P, 2, QC], F32, tag="pss", name="pss")
                    # two row-tiled 64-contraction matmuls fill both halves
                    nc.tensor.matmul(pss[:, 0, :], kT_sb[p][0:DK, ksl],
                                     qT_sb[p][0:DK, qsl])
                    nc.tensor.matmul(pss[:, 1, :], kT_sb[p][DK:P, ksl],
                                     qT_sb[p][DK:P, qsl])
                    pt = ppool.tile([P, 2, QC], st, tag="pt", name="pt")
                    nc.scalar.activation(pt[:], pss[:],
                                         mybir.ActivationFunctionType.Exp,
                                         scale=inv_sqrt)
                    if mode == TRI:
                        r = arg
                        if r > 0:
                            nc.vector.memset(pt[:, :, 0:r * P], 0.0)
                        nc.vector.tensor_mul(pt[:, :, r * P:(r + 1) * P],
                                             pt[:, :, r * P:(r + 1) * P],
                                             tri2[:])
                    elif mode == GEN:
                        for j in range(2):
                            nc.vector.tensor_mul(pt[:, j, :], pt[:, j, :],
                                                 gen_sb[:, arg, :])
                    for j in range(2):
                        h = 2 * p + j
                        nc.tensor.matmul(
                            po[j][:],
                            v_aug[:, ikb, h * (DK + 1):(h + 1) * (DK + 1)],
                            pt[:, j, :],
                            start=(gi_ == 0), stop=(gi_ == nblk - 1))
                    for f in inject.pop_some(pair=p, gi=gi_, nblk=nblk):
                        f()
                # hide the normalize latency under reserved projection work
                for f in inject.pop_boundary():
                    f()
                # normalize: cu rows j*64.. of tile column p
                rec1 = [work.tile([1, QC], F32, tag=f"rec1{j}", name="rec1")
                        for j in range(2)]
                recb = [work.tile([DK, QC], F32, tag=f"recb{j}", name="recb")
                        for j in range(2)]
                for j in range(2):
                    nc.vector.reciprocal_approx_fast(rec1[j][:],
                                                     po[j][DK:DK + 1, :])
                for j in range(2):
                    nc.gpsimd.partition_broadcast(recb[j][:], rec1[j][:])
                for j in range(2):
                    nc.vector.tensor_mul(cu[j * DK:(j + 1) * DK, p, :],
                                         po[j][0:DK, :], recb[j][:])
                # ship this pair's 128 concatT rows as soon as they're ready
                nc.sync.dma_start(
                    out=cc_in[c][:, p * P:(p + 1) * P, :]
                        .rearrange("j p q -> p j q"),
                    in_=cu[:, p, :].rearrange("p (j q) -> p j q", j=N_CORES))
            if collective:
                nc.gpsimd.collective_compute(
                    "AllToAll", mybir.AluOpType.bypass,
                    replica_groups=[list(range(N_CORES))],
                    ins=[cc_in[c][:].opt()],
                    outs=[cc_out[c][:].opt()])
            else:
                nc.sync.dma_start(out=cc_out[c][:], in_=cc_in[c][:])

        class Injector:
            """Spreads a chunk's projection units across attention blocks,
            holding 2 back per pair boundary to hide the normalize latency."""

            def __init__(self, units, n_boundaries=2, per_boundary=2):
                self.units = list(units)
                self.reserve = min(len(self.units),
                                   n_boundaries * per_boundary)
                self.per_boundary = per_boundary
                self.spread = len(self.units) - self.reserve
                self.emitted = 0
                self.seen = 0

            def pop_some(self, pair, gi, nblk):
                total_slots = 2 * nblk
                self.seen += 1
                want = (self.seen * self.spread + total_slots - 1) // total_slots
                out = []
                while (self.units and self.emitted < want
                       and len(self.units) > self.reserve):
                    out.append(self.units.pop(0))
                    self.emitted += 1
                return out

            def pop_boundary(self):
                out = []
                for _ in range(min(self.per_boundary, self.reserve,
                                   len(self.units))):
                    out.append(self.units.pop(0))
                    self.reserve -= 1
                return out

        # ---- output projection (after AllToAll of chunk c) ----
        def outproj_chunk(c):
            # rb[p, t, i, q]: slot i = rank i's dh slice (i<4: batch 0 dims,
            # i>=4: batch 1), t = 128-row half of that slice
            rb = rbpool.tile([P, NDH, N_CORES, QB], st, tag="rb",
                             name=f"rb{c}")
            for t in range(NDH):
                nc.sync.dma_start(
                    out=rb[:, t, :, :],
                    in_=cc_out[c][:, t * P:(t + 1) * P, :]
                        .rearrange("i p q -> p i q"))
            ys = ypool.tile([P, DM], F32, tag="ys", name=f"ys{c}")
            for half in range(DM // QC):
                y2 = ps_p.tile([P, QC], F32, tag="ps_w", name="ps_y")
                msl = slice(half * QC, (half + 1) * QC)
                for u in range(NDE):
                    # batch 0 -> psum rows 0:64, batch 1 -> rows 64:128
                    # (col-tiled pair, auto tile_position from base partition)
                    nc.tensor.matmul(y2[0:QB, :], rb[:, u % 2, u // 2, :],
                                     wo_sb[u][:, msl],
                                     start=(u == 0), stop=(u == NDE - 1))
                    nc.tensor.matmul(y2[QB:2 * QB, :],
                                     rb[:, u % 2, 4 + u // 2, :],
                                     wo_sb[u][:, msl],
                                     start=(u == 0), stop=(u == NDE - 1))
                nc.vector.tensor_add(ys[:, msl], y2[:], bo_b[:, msl])
            nc.sync.dma_start(out=y_out[c * P:(c + 1) * P, :], in_=ys[:])

        # ---- phase schedule ----
        for f in proj_units(0):
            f()
        for c in range(NQC):
            if c + 1 < NQC:
                load_x(c + 1)
                inj = Injector(proj_units(c + 1))
            else:
                inj = Injector([])
            attention_chunk(c, inj)
            # chunk c's AllToAll overlaps later compute; emit each output
            # projection as soon as its collective is safely complete
            if c == 2:
                outproj_chunk(0)
            elif c == 3:
                for cc in range(1, NQC):
                    outproj_chunk(cc)

    nc.compile()
    return nc


# ------------------------------------------------------------------
_CACHE = {}


def _get_compiled(plan_key, blocks, n_gen):
    if plan_key not in _CACHE:
        nc = build_mha(blocks, n_gen)
        nc.m = get_hw_module(nc.m)
        _CACHE[plan_key] = nc
    return _CACHE[plan_key]


def make_in_maps(q, k, v, mask, W_in, b_in, Wq, bq, Wk, bk, Wv, bv, Wo, bo,
                 blocks=None, n_gen=None, gen_tiles=None):
    import ml_dtypes
    bf = ml_dtypes.bfloat16
    if blocks is None:
        blocks, n_gen, gen_tiles = make_plan(mask)
    tb = lambda a: np.ascontiguousarray(np.asarray(a).T).astype(bf)
    cb = lambda a: np.ascontiguousarray(np.asarray(a)).astype(bf)
    in_maps = []
    for c in range(N_CORES):
        b, g = c // 4, c % 4
        sl = slice(g * DH, (g + 1) * DH)
        mt = (gen_tiles[b] if n_gen else
              np.zeros((1, KB, QC), np.int32))
        in_maps.append({
            "qT": tb(q[b]), "kT": tb(k[b]), "vT": tb(v[b]),
            "w_inT": tb(W_in),
            "wq": cb(Wq[:, sl]),
            "wk": cb(Wk[:, sl]),
            "wv": cb(Wv[:, sl]),
            "wo": cb(Wo),
            "b_in": cb(b_in),
            "bq": cb(bq[sl]),
            "bk": cb(bk[sl]),
            "bv": cb(bv[sl]),
            "bo": np.asarray(bo, np.float32),
            "m_tiles": mt,
        })
    return in_maps, blocks, n_gen


def assemble(results):
    out = np.empty((B, S, DM), np.float32)
    for r in range(N_CORES):
        y = results[r]["y_out"]               # [NQC*128, DM]
        for c in range(NQC):
            rows = slice(c * QC + r * QB, c * QC + (r + 1) * QB)
            out[0, rows, :] = y[c * P:c * P + QB, :]
            out[1, rows, :] = y[c * P + QB:(c + 1) * P, :]
    return out


def kernel(**inputs):
    mask = inputs["mask"]
    blocks, n_gen, gen_tiles = make_plan(np.asarray(mask))
    plan_key = (str(blocks), n_gen)
    nc = _get_compiled(plan_key, blocks, n_gen)
    in_maps, _, _ = make_in_maps(
        inputs["q"], inputs["k"], inputs["v"], mask,
        inputs["W_in"], inputs["b_in"], inputs["Wq"], inputs["bq"],
        inputs["Wk"], inputs["bk"], inputs["Wv"], inputs["bv"],
        inputs["Wo"], inputs["bo"],
        blocks=blocks, n_gen=n_gen, gen_tiles=gen_tiles)
    res = bass_utils.run_bass_kernel_spmd(nc, in_maps,
                                          core_ids=list(range(N_CORES)))
    return assemble(res.results)


# revision 21
# speedup vs baseline: 2.8596x; 1.0054x over previous
"""Trainium2 Bass kernel for nn_MultiHeadAttention_72069551227273.

Reference computation (B=2, S=2048, D_MODEL=D_EMB=1024, H=16, d_k=64):
    q_p = q @ W_in + b_in                    (shared input projection)
    qh  = heads(q_p @ Wq + bq)               (per-head projections)
    s   = qh @ kh^T / sqrt(d_k), causal-masked softmax
    out = (attn @ vh, concat heads) @ Wo + bo

Sharding: 8 cores = 2 (batch) x 4 (head groups of 4 heads / 256 emb cols).
Per core the input and head projections are fused algebraically:
    Q = q @ (W_in @ Wq_slice) + (b_in @ Wq_slice + bq_slice)
with the weight combine computed on device.  All activations that feed
matmuls are bf16; the host pre-casts inputs/weights to bf16 so every load
is a plain HWDGE DMA (no software-DGE cast descriptors).  Q/K live in
SBUF transposed ([feature, seq]); V is produced directly in natural
[seq, feature] orientation (stationary = x^T tile), so no PE transposes
are needed anywhere.  Softmax skips max-subtraction (scores are O(1))
and gets its denominator from a ones-column appended to V.  Mask
structure is resolved at trace time (skip / full / triangular / general
blocks); score blocks for a pair of heads are computed by two row-tiled
matmuls (64-row contraction each) sharing one PSUM tile and one Exp
activation instruction.

Collective: instead of reduce-scattering the f32 output partials
(8.4 MB/core), each core ships its bf16 transposed concat slice
[256, 512] per q-chunk through a single 8-rank AllToAll (256 KB/rank):
global rank r receives, for its own 64-row q-block of every chunk, the
full 1024-dim concat rows of BOTH batches.  The output projection then
runs fully local (contraction over all 1024 emb dims, the two batches
col-tiled into one PSUM tile), and per-chunk collectives overlap the
remaining attention compute.
"""

import sys

sys.path.append("/opt/trn_rl_repo")

import math
from contextlib import ExitStack

import numpy as np

import concourse.bass as bass
import concourse.bacc as bacc
import concourse.mybir as mybir
import concourse.tile as tile
from concourse import bass_utils
from concourse.bass_interp import get_hw_module

# problem dims
B, S, DM, DE, H, DK = 2, 2048, 1024, 1024, 16, 64
N_CORES = 8
P = 128                      # partitions
QC = 512                     # q chunk (psum bank width in fp32)
KB = 128                     # k block (scores^T partition block)
NQC, NKB, NDM, NDE = S // QC, S // KB, DM // P, DE // P
DH = DE // 4                 # per-core emb slice (256)
NDH = DH // P                # 2
HLOC = DH // DK              # heads per core (4)
QB = QC // N_CORES           # per-rank q rows per chunk (64)

F32 = mybir.dt.float32
BF16 = mybir.dt.bfloat16

FULL, TRI, GEN, SKIP = 0, 1, 2, 3


def make_plan(mask_np, s=S, qc=QC, kb=KB):
    """Classify scores^T blocks [kb x qc] from the (B, S, S) 0/1 mask.

    Returns (blocks, n_gen_tiles, gen_tiles_per_batch):
      blocks[iqc] = list of (ikb, mode, arg)
    """
    nqc, nkb = s // qc, s // kb
    m = np.asarray(mask_np) != 0          # [B, S(q), S(k)] True = attend
    tril = np.tril(np.ones((s, s), bool))
    causal = all(np.array_equal(m[b], tril) for b in range(m.shape[0]))
    blocks = []
    if causal:
        for iqc in range(nqc):
            row = []
            for ikb in range(nkb):
                if (ikb + 1) * kb <= iqc * qc:
                    row.append((ikb, FULL, 0))
                elif ikb * kb < (iqc + 1) * qc:
                    row.append((ikb, TRI, (ikb * kb - iqc * qc) // kb))
                # else fully masked -> skip
            blocks.append(row)
        return blocks, 0, None

    # general path: per-block classification, unioned across batches
    nb = m.shape[0]
    # every query row must attend to >= 1 key (else softmax semantics differ)
    assert m.any(axis=-1).all(), "fully-masked query rows unsupported"
    gen_tiles = [[] for _ in range(nb)]
    for iqc in range(nqc):
        row = []
        for ikb in range(nkb):
            sub = m[:, iqc * qc:(iqc + 1) * qc, ikb * kb:(ikb + 1) * kb]
            if sub.all():
                row.append((ikb, FULL, 0))
            elif not sub.any():
                continue
            else:
                idx = len(gen_tiles[0])
                for b in range(nb):
                    gen_tiles[b].append(sub[b].T.astype(np.int32))  # [kb, qc]
                row.append((ikb, GEN, idx))
        blocks.append(row)
    n_gen = len(gen_tiles[0])
    gt = [np.stack(g) if n_gen else np.zeros((1, kb, qc), np.int32)
          for g in gen_tiles]
    return blocks, n_gen, gt


def build_mha(blocks, n_gen, *, collective=True):
    """Trace the per-core MHA program."""
    st = BF16
    inv_sqrt = 1.0 / math.sqrt(DK)

    nc = bacc.Bacc("TRN2", target_bir_lowering=False, debug=False,
                   num_devices=N_CORES)

    # ---- kernel I/O (per core; bf16 pre-cast on host) ----
    qT = nc.dram_tensor("qT", [DM, S], st, kind="ExternalInput")
    kT = nc.dram_tensor("kT", [DM, S], st, kind="ExternalInput")
    vT = nc.dram_tensor("vT", [DM, S], st, kind="ExternalInput")
    w_inT = nc.dram_tensor("w_inT", [DE, DM], st, kind="ExternalInput")
    wq = nc.dram_tensor("wq", [DE, DH], st, kind="ExternalInput")
    wk = nc.dram_tensor("wk", [DE, DH], st, kind="ExternalInput")
    wv = nc.dram_tensor("wv", [DE, DH], st, kind="ExternalInput")
    wo = nc.dram_tensor("wo", [DE, DM], st, kind="ExternalInput")   # full Wo
    b_in = nc.dram_tensor("b_in", [DE], st, kind="ExternalInput")
    bq = nc.dram_tensor("bq", [DH], st, kind="ExternalInput")
    bk = nc.dram_tensor("bk", [DH], st, kind="ExternalInput")
    bv = nc.dram_tensor("bv", [DH], st, kind="ExternalInput")
    bo = nc.dram_tensor("bo", [DM], F32, kind="ExternalInput")
    m_tiles = nc.dram_tensor("m_tiles", [max(n_gen, 1), KB, QC],
                             mybir.dt.int32, kind="ExternalInput")
    # rows (chunk, batch, 64): chunk c gives q rows c*512 + rank*64 of both b
    y_out = nc.dram_tensor("y_out", [NQC * P, DM], F32, kind="ExternalOutput")

    # per-chunk AllToAll buffers: shard j = my concatT slice [256, 64] for
    # global rank j's q-block.  After A2A, slot i = rank i's dh slice for MY
    # q-block; slots 0-3 stack batch-0's 1024 concat dims, 4-7 batch-1's.
    cc_in = [nc.dram_tensor(f"cc_in{c}", [N_CORES, DH, QB], st)
             for c in range(NQC)]
    cc_out = [nc.dram_tensor(f"cc_out{c}", [N_CORES, DH, QB], st)
              for c in range(NQC)]

    with tile.TileContext(nc) as tc, ExitStack() as ex:
        persist = ex.enter_context(tc.tile_pool(name="persist", bufs=1))
        work = ex.enter_context(tc.tile_pool(name="work", bufs=3))
        ps_s = ex.enter_context(tc.tile_pool(name="ps_s", bufs=2, space="PSUM"))
        ps_o = ex.enter_context(tc.tile_pool(name="ps_o", bufs=1, space="PSUM"))
        ps_p = ex.enter_context(tc.tile_pool(name="ps_p", bufs=2, space="PSUM"))
        xpool = ex.enter_context(tc.tile_pool(name="xpool", bufs=2))
        wscope = ExitStack()
        wpool = wscope.enter_context(tc.tile_pool(name="wpool", bufs=1))

        # ---- constants ----
        # tri2[k, j, q] = 1.0 where k <= q (keep), else 0; j = head-in-pair
        tri2 = persist.tile([P, 2, P], st, tag="tri2", name="tri2")
        nc.gpsimd.memset(tri2[:], 0.0)
        for j in range(2):
            nc.gpsimd.affine_select(out=tri2[:, j, :], in_=tri2[:, j, :],
                                    compare_op=mybir.AluOpType.is_gt,
                                    fill=1.0, base=0,
                                    pattern=[[-1, P]], channel_multiplier=1)
        ones1 = persist.tile([1, P], st, tag="ones1", name="ones1")
        nc.gpsimd.memset(ones1[:], 1.0)
        gen_sb = None
        if n_gen:
            gen_sb = persist.tile([P, n_gen, QC], st, tag="gen", name="gen")
            gi = persist.tile([P, n_gen, QC], mybir.dt.int32, tag="gen_i",
                              name="gen_i")
            nc.sync.dma_start(gi[:], m_tiles[:].rearrange("n p q -> p n q"))
            for i in range(n_gen):
                nc.vector.tensor_copy(gen_sb[:, i, :], gi[:, i, :])

        # ---- load weights (plain HWDGE; everything already bf16) ----
        w_inT_b = wpool.tile([P, NDE, DM], st, tag="w_inT", name="w_inT_b")
        hd = NDE // 2
        nc.sync.dma_start(out=w_inT_b[:, 0:hd, :],
                          in_=w_inT[0:hd * P, :].rearrange("(u p) m -> p u m", p=P))
        nc.sync.dma_start(out=w_inT_b[:, hd:NDE, :],
                          in_=w_inT[hd * P:, :].rearrange("(u p) m -> p u m", p=P))
        w_inT_sb = [w_inT_b[:, u, :] for u in range(NDE)]
        w_sb = {}
        for name, w in (("q", wq), ("k", wk), ("v", wv)):
            wb = wpool.tile([P, NDE, DH], st, tag=f"w{name}", name=f"w{name}_b")
            nc.sync.dma_start(out=wb[:], in_=w[:].rearrange("(u p) d -> p u d", p=P))
            w_sb[name] = [wb[:, u, :] for u in range(NDE)]
        b_inT = wpool.tile([P, NDE], st, tag="b_inT", name="b_inT")
        nc.sync.dma_start(out=b_inT[:], in_=b_in[:].rearrange("(t p) -> p t", p=P))

        # ---- input staging (x(0) queued right behind the combine weights,
        # so it lands while the weight combine computes) ----
        xtiles = {}

        def load_x(c):
            for name, xdram in (("q", qT), ("k", kT), ("v", vT)):
                xt = xpool.tile([P, NDM, QC], st, tag=f"x{name}",
                                name=f"x{name}{c}")
                nc.sync.dma_start(
                    out=xt[:],
                    in_=xdram[:, c * QC:(c + 1) * QC]
                        .rearrange("(u p) s -> p u s", p=P))
                xtiles[(name, c)] = xt

        load_x(0)

        # ---- combine weights: Wc_x = W_in @ Wx (+ bias fold) ----
        wc = {}
        bc = {}
        for name in ("q", "k", "v"):
            wc[name] = [persist.tile([P, DH], st, tag=f"wc{name}{t}",
                                     name=f"wc{name}{t}") for t in range(NDM)]
            for t in range(NDM):
                ps = ps_p.tile([P, DH], F32, tag="ps_w", name="ps_w")
                for u in range(NDE):
                    nc.tensor.matmul(
                        ps[:], w_inT_sb[u][:, t * P:(t + 1) * P],
                        w_sb[name][u][:],
                        start=(u == 0), stop=(u == NDE - 1))
                nc.vector.tensor_copy(wc[name][t][:], ps[:])
        # Q/K bias (column layout, added per-partition after projection)
        for name, bvec in (("q", bq), ("k", bk)):
            bxT = wpool.tile([P, NDH], st, tag=f"bxT{name}", name=f"bxT{name}")
            nc.sync.dma_start(out=bxT[:], in_=bvec[:].rearrange("(t p) -> p t", p=P))
            bc[name] = persist.tile([P, NDH], F32, tag=f"bc{name}", name=f"bc{name}")
            for t in range(NDH):
                ps = ps_p.tile([P, 1], F32, tag="ps_w", name="ps_b")
                for u in range(NDE):
                    nc.tensor.matmul(
                        ps[:], w_sb[name][u][:, t * P:(t + 1) * P],
                        b_inT[:, u:u + 1],
                        start=(u == 0), stop=(u == NDE - 1))
                nc.vector.tensor_add(bc[name][:, t:t + 1], ps[:], bxT[:, t:t + 1])
        # V bias (row layout, accumulated into the psum by a ones matmul)
        bv_row = wpool.tile([1, DH], st, tag="bv_row", name="bv_row")
        nc.sync.dma_start(out=bv_row[:], in_=bv[:].unsqueeze(0))
        bcv = persist.tile([1, DH], st, tag="bcv", name="bcv")
        psb = ps_p.tile([1, DH], F32, tag="ps_w", name="ps_b")
        for u in range(NDE):
            nc.tensor.matmul(psb[:], b_inT[:, u:u + 1], w_sb["v"][u][:],
                             start=(u == 0), stop=(u == NDE - 1))
        nc.vector.tensor_add(bcv[:], psb[:], bv_row[:])

        # late-needed weights, emitted after the x(0) loads in queue order
        wo_b = persist.tile([P, NDE, DM], st, tag="wo", name="wo_b")
        nc.sync.dma_start(out=wo_b[:], in_=wo[:].rearrange("(u p) m -> p u m", p=P))
        wo_sb = [wo_b[:, u, :] for u in range(NDE)]
        bo_b = persist.tile([P, DM], F32, tag="bo_b", name="bo_b")
        nc.sync.dma_start(out=bo_b[:],
                          in_=bo[:].unsqueeze(0).broadcast_to([P, DM]))

        wscope.close()   # frees weight-staging SBUF for the streaming pools
        ppool = ex.enter_context(tc.tile_pool(name="ppool", bufs=3))
        cupool = ex.enter_context(tc.tile_pool(name="cupool", bufs=2))
        rbpool = ex.enter_context(tc.tile_pool(name="rbpool", bufs=2))
        ypool = ex.enter_context(tc.tile_pool(name="ypool", bufs=2))

        # ---- persistent activation tiles ----
        qT_sb = [persist.tile([P, S], st, tag=f"qT{t}", name=f"qT{t}")
                 for t in range(NDH)]
        kT_sb = [persist.tile([P, S], st, tag=f"kT{t}", name=f"kT{t}")
                 for t in range(NDH)]
        # V natural [key, dv] with a ones column per head: head h occupies
        # cols h*65 .. h*65+64 (65th col = softmax denominator ones)
        v_aug = persist.tile([P, NKB, HLOC * (DK + 1)], st, tag="vaug",
                             name="vaug")
        for h in range(HLOC):
            nc.gpsimd.memset(v_aug[:, :, h * (DK + 1) + DK:
                                    h * (DK + 1) + DK + 1], 1.0)

        # ---- projection units (Q/K transposed, V natural), as fine-grained
        # micro-steps (~2 matmuls each) so they can pace-fill PE idle time
        # between attention blocks ----
        def proj_qk_steps(name, c, t):
            dst = qT_sb if name == "q" else kT_sb
            box = {}

            def mms(us):
                def f():
                    xs = xtiles[(name, c)]
                    if us[0] == 0:
                        box["ps"] = ps_p.tile([P, QC], F32, tag="ps_w",
                                              name="ps_w")
                    for u in us:
                        nc.tensor.matmul(
                            box["ps"][:], wc[name][u][:, t * P:(t + 1) * P],
                            xs[:, u, :],
                            start=(u == 0), stop=(u == NDM - 1))
                return f

            def fin():
                nc.vector.tensor_scalar_add(
                    dst[t][:, c * QC:(c + 1) * QC], box["ps"][:],
                    bc[name][:, t:t + 1])

            return [mms((0, 1)), mms((2, 3)), mms((4, 5)), mms((6, 7)), fin]

        def proj_v_steps(c, jb):
            ikb = c * (QC // P) + jb
            box = {}

            def mms(us):
                def f():
                    xs = xtiles[("v", c)]
                    if us[0] == 0:
                        box["ps"] = ps_p.tile([P, DH], F32, tag="ps_w",
                                              name="ps_w")
                    for u in us:
                        nc.tensor.matmul(
                            box["ps"][:], xs[:, u, jb * P:(jb + 1) * P],
                            wc["v"][u][:],
                            start=(u == 0), stop=False)
                return f

            def fin():
                nc.tensor.matmul(box["ps"][:], ones1[:], bcv[:],
                                 start=False, stop=True)
                nc.vector.tensor_copy(
                    v_aug[:, ikb, :].rearrange("p (h u) -> p h u",
                                               h=HLOC)[:, :, 0:DK],
                    box["ps"][:].rearrange("p (h u) -> p h u", h=HLOC))

            return [mms((0, 1)), mms((2, 3)), mms((4, 5)), mms((6, 7)), fin]

        def proj_units(c):
            steps = []
            for t in range(NDH):
                steps += proj_qk_steps("q", c, t)
                steps += proj_qk_steps("k", c, t)
            for j in range(QC // P):
                steps += proj_v_steps(c, j)
            return steps

        # ---- attention ----
        def attention_chunk(c, inject):
            """inject: iterator of closures run between block iterations."""
            blist = blocks[c]
            qsl = slice(c * QC, (c + 1) * QC)
            cu = cupool.tile([P, NDH, QC], st, tag="cu", name=f"cu{c}")
            for p in range(NDH):        # head pair p = heads (2p, 2p+1)
                po = [ps_o.tile([DK + 1, QC], F32, tag=f"po{j}", name=f"po{j}")
                      for j in range(2)]
                nblk = len(blist)
                for gi_, (ikb, mode, arg) in enumerate(blist):
                    ksl = slice(ikb * KB, (ikb + 1) * KB)
                    pss = ps_s.tile([P, 2, QC], F32, tag="pss", name="pss")
                    # two row-tiled 64-contraction matmuls fill both halves
                    nc.tensor.matmul(pss[:, 0, :], kT_sb[p][0:DK, ksl],
                                     qT_sb[p][0:DK, qsl])
                    nc.tensor.matmul(pss[:, 1, :], kT_sb[p][DK:P, ksl],
                                     qT_sb[p][DK:P, qsl])
                    pt = ppool.tile([P, 2, QC], st, tag="pt", name="pt")
                    # TRI blocks: columns left of the diagonal sub-block are
                    # fully masked — skip their exp, just zero them.
                    r0 = arg * P if mode == TRI else 0
                    nc.scalar.activation(pt[:, :, r0:], pss[:, :, r0:],
                                         mybir.ActivationFunctionType.Exp,
                                         scale=inv_sqrt)
                    if mode == TRI:
                        r = arg
                        if r > 0:
                            nc.vector.memset(pt[:, :, 0:r * P], 0.0)
                        nc.vector.tensor_mul(pt[:, :, r * P:(r + 1) * P],
                                             pt[:, :, r * P:(r + 1) * P],
                                             tri2[:])
                    elif mode == GEN:
                        for j in range(2):
                            nc.vector.tensor_mul(pt[:, j, :], pt[:, j, :],
                                                 gen_sb[:, arg, :])
                    for j in range(2):
                        h = 2 * p + j
                        nc.tensor.matmul(
                            po[j][:],
                            v_aug[:, ikb, h * (DK + 1):(h + 1) * (DK + 1)],
                            pt[:, j, :],
                            start=(gi_ == 0), stop=(gi_ == nblk - 1))
                    for f in inject.pop_some(pair=p, gi=gi_, nblk=nblk):
                        f()
                # hide the normalize latency under reserved projection work
                for f in inject.pop_boundary():
                    f()
                # normalize: cu rows j*64.. of tile column p
                rec1 = [work.tile([1, QC], F32, tag=f"rec1{j}", name="rec1")
                        for j in range(2)]
                recb = [work.tile([DK, QC], F32, tag=f"recb{j}", name="recb")
                        for j in range(2)]
                for j in range(2):
                    nc.vector.reciprocal_approx_fast(rec1[j][:],
                                                     po[j][DK:DK + 1, :])
                for j in range(2):
                    nc.gpsimd.partition_broadcast(recb[j][:], rec1[j][:])
                for j in range(2):
                    nc.vector.tensor_mul(cu[j * DK:(j + 1) * DK, p, :],
                                         po[j][0:DK, :], recb[j][:])
                # ship this pair's 128 concatT rows as soon as they're ready
                nc.sync.dma_start(
                    out=cc_in[c][:, p * P:(p + 1) * P, :]
                        .rearrange("j p q -> p j q"),
                    in_=cu[:, p, :].rearrange("p (j q) -> p j q", j=N_CORES))
            if collective:
                nc.gpsimd.collective_compute(
                    "AllToAll", mybir.AluOpType.bypass,
                    replica_groups=[list(range(N_CORES))],
                    ins=[cc_in[c][:].opt()],
                    outs=[cc_out[c][:].opt()])
            else:
                nc.sync.dma_start(out=cc_out[c][:], in_=cc_in[c][:])

        class Injector:
            """Spreads a chunk's projection units across attention blocks,
            holding 2 back per pair boundary to hide the normalize latency."""

            def __init__(self, units, n_boundaries=2, per_boundary=2):
                self.units = list(units)
                self.reserve = min(len(self.units),
                                   n_boundaries * per_boundary)
                self.per_boundary = per_boundary
                self.spread = len(self.units) - self.reserve
                self.emitted = 0
                self.seen = 0

            def pop_some(self, pair, gi, nblk):
                total_slots = 2 * nblk
                self.seen += 1
                want = (self.seen * self.spread + total_slots - 1) // total_slots
                out = []
                while (self.units and self.emitted < want
                       and len(self.units) > self.reserve):
                    out.append(self.units.pop(0))
                    self.emitted += 1
                return out

            def pop_boundary(self):
                out = []
                for _ in range(min(self.per_boundary, self.reserve,
                                   len(self.units))):
                    out.append(self.units.pop(0))
                    self.reserve -= 1
                return out

        # ---- output projection (after AllToAll of chunk c) ----
        def outproj_rb(c):
            # rb[p, t, i, q]: slot i = rank i's dh slice (i<4: batch 0 dims,
            # i>=4: batch 1), t = 128-row half of that slice
            rb = rbpool.tile([P, NDH, N_CORES, QB], st, tag="rb",
                             name=f"rb{c}")
            for t in range(NDH):
                nc.sync.dma_start(
                    out=rb[:, t, :, :],
                    in_=cc_out[c][:, t * P:(t + 1) * P, :]
                        .rearrange("i p q -> p i q"))
            return rb

        def outproj_steps(c, rb):
            box = {}

            def mms(half, us):
                def f():
                    if us[0] == 0:
                        box[half] = ps_p.tile([P, QC], F32, tag="ps_w",
                                              name="ps_y")
                    y2 = box[half]
                    msl = slice(half * QC, (half + 1) * QC)
                    for u in us:
                        # batch 0 -> psum rows 0:64, batch 1 -> rows 64:128
                        # (col-tiled pair, auto tile_position)
                        nc.tensor.matmul(y2[0:QB, :], rb[:, u % 2, u // 2, :],
                                         wo_sb[u][:, msl],
                                         start=(u == 0), stop=(u == NDE - 1))
                        nc.tensor.matmul(y2[QB:2 * QB, :],
                                         rb[:, u % 2, 4 + u // 2, :],
                                         wo_sb[u][:, msl],
                                         start=(u == 0), stop=(u == NDE - 1))
                return f

            def fin(half):
                def f():
                    if half == 0:
                        box["ys"] = ypool.tile([P, DM], F32, tag="ys",
                                               name=f"ys{c}")
                    msl = slice(half * QC, (half + 1) * QC)
                    nc.vector.tensor_add(box["ys"][:, msl], box[half][:],
                                         bo_b[:, msl])
                    if half == DM // QC - 1:
                        nc.sync.dma_start(out=y_out[c * P:(c + 1) * P, :],
                                          in_=box["ys"][:])
                return f

            steps = []
            for half in range(DM // QC):
                steps += [mms(half, (0, 1)), mms(half, (2, 3)),
                          mms(half, (4, 5)), mms(half, (6, 7)), fin(half)]
            return steps

        def outproj_chunk(c):
            rb = outproj_rb(c)
            for f in outproj_steps(c, rb):
                f()

        # ---- phase schedule ----
        for f in proj_units(0):
            f()
        for c in range(NQC):
            if c + 1 < NQC:
                load_x(c + 1)
                inj = Injector(proj_units(c + 1), per_boundary=4)
            else:
                # fill chunk 3's PE idle time with outproj(0) — its AllToAll
                # completed long before chunk 3's attention starts
                rb0 = outproj_rb(0)
                inj = Injector(outproj_steps(0, rb0), per_boundary=2)
            attention_chunk(c, inj)
        # remaining output projections; (1) and (2) overlap the final AllToAll
        for cc in range(1, NQC):
            outproj_chunk(cc)

    nc.compile()
    return nc


# ------------------------------------------------------------------
_CACHE = {}


def _get_compiled(plan_key, blocks, n_gen):
    if plan_key not in _CACHE:
        nc = build_mha(blocks, n_gen)
        nc.m = get_hw_module(nc.m)
        _CACHE[plan_key] = nc
    return _CACHE[plan_key]


def make_in_maps(q, k, v, mask, W_in, b_in, Wq, bq, Wk, bk, Wv, bv, Wo, bo,
                 blocks=None, n_gen=None, gen_tiles=None):
    import ml_dtypes
    bf = ml_dtypes.bfloat16
    if blocks is None:
        blocks, n_gen, gen_tiles = make_plan(mask)
    tb = lambda a: np.ascontiguousarray(np.asarray(a).T).astype(bf)
    cb = lambda a: np.ascontiguousarray(np.asarray(a)).astype(bf)
    in_maps = []
    for c in range(N_CORES):
        b, g = c // 4, c % 4
        sl = slice(g * DH, (g + 1) * DH)
        mt = (gen_tiles[b] if n_gen else
              np.zeros((1, KB, QC), np.int32))
        in_maps.append({
            "qT": tb(q[b]), "kT": tb(k[b]), "vT": tb(v[b]),
            "w_inT": tb(W_in),
            "wq": cb(Wq[:, sl]),
            "wk": cb(Wk[:, sl]),
            "wv": cb(Wv[:, sl]),
            "wo": cb(Wo),
            "b_in": cb(b_in),
            "bq": cb(bq[sl]),
            "bk": cb(bk[sl]),
            "bv": cb(bv[sl]),
            "bo": np.asarray(bo, np.float32),
            "m_tiles": mt,
        })
    return in_maps, blocks, n_gen


def assemble(results):
    out = np.empty((B, S, DM), np.float32)
    for r in range(N_CORES):
        y = results[r]["y_out"]               # [NQC*128, DM]
        for c in range(NQC):
            rows = slice(c * QC + r * QB, c * QC + (r + 1) * QB)
            out[0, rows, :] = y[c * P:c * P + QB, :]
            out[1, rows, :] = y[c * P + QB:(c + 1) * P, :]
    return out


def kernel(**inputs):
    mask = inputs["mask"]
    blocks, n_gen, gen_tiles = make_plan(np.asarray(mask))
    plan_key = (str(blocks), n_gen)
    nc = _get_compiled(plan_key, blocks, n_gen)
    in_maps, _, _ = make_in_maps(
        inputs["q"], inputs["k"], inputs["v"], mask,
        inputs["W_in"], inputs["b_in"], inputs["Wq"], inputs["bq"],
        inputs["Wk"], inputs["bk"], inputs["Wv"], inputs["bv"],
        inputs["Wo"], inputs["bo"],
        blocks=blocks, n_gen=n_gen, gen_tiles=gen_tiles)
    res = bass_utils.run_bass_kernel_spmd(nc, in_maps,
                                          core_ids=list(range(N_CORES)))
    return assemble(res.results)
